# revision 9
# baseline (speedup 1.0000x reference)
"""NT-Xent (SimCLR) contrastive loss kernel for Trainium2, 8 NeuronCores.

Reference computation (B=4096, D=256, T=0.5):
    out  = concat(out_1, out_2)              # [8192, 256]
    sim  = exp(out @ out.T / T)              # [8192, 8192]
    diag = exp(sum(out*out, -1) / T)
    row_sum = sim.sum(-1) - diag
    pos  = exp(sum(out_1*out_2, -1) / T), duplicated
    loss = mean(-log(pos / row_sum)) = mean(log(row_sum) - 2*sum(out_1*out_2, -1))

Sharding: data-parallel over the 8192 rows of sim; core c owns rows
[c*1024, (c+1)*1024). Host-side prep (part of the sharding/layout
strategy): concatenate out -> bf16 copy `o16` plus each core's own f32 row
block and its positive-pair block. Each core then:
  1. builds the full out.T [256, 8192] bf16 in SBUF with DMA-xbar
     transposes (no tensor-engine or gpsimd time),
  2. computes its row-block scores with bf16 matmuls into f32 PSUM,
  3. applies exp(2x) on the scalar engine with fused row-sum accumulation,
  4. computes diag/pos in f32 from its natural-layout blocks and reduces
     its local loss partial with a ones-matmul.
The host sums the 8 partial outputs.

Numerics: row norms ||out_i||^2 ~ 256, so diag = exp(~512) = inf in f32 and
row_sum = inf - inf = nan -> loss = nan, exactly as the reference produces
(HW-verified: ACT exp -> inf, DVE inf-inf -> nan, ACT ln(nan) -> nan). The
bf16 score matmul cannot disturb the nan path; diag/pos terms are computed
in f32.
"""

import os
import sys

for _p in ("/opt/trn_rl_repo", "/root/.axon_site/_ro/trn_rl_repo"):
    if os.path.isdir(_p) and _p not in sys.path:
        sys.path.insert(0, _p)

import ml_dtypes
import numpy as np

import concourse.bass as bass
import concourse.mybir as mybir
from concourse import bacc
from concourse.bass_utils import run_bass_kernel_spmd
from concourse.tile import TileContext

P = 128
D = 256
B = 4096
NT = 2 * B  # 8192 total rows
NCORES = 8
R = NT // NCORES  # 1024 rows per core
MT = R // P  # 8 m-tiles per core
KCH = D // P  # 2 contraction chunks
GRP = 2048  # psum group width (4 banks f32)
NG = NT // GRP  # 4 groups
NBLK = 512  # matmul free dim
JPG = GRP // NBLK  # 4 matmul blocks per group
F32 = mybir.dt.float32
BF16 = mybir.dt.bfloat16

_CACHE: dict = {}


def _build():
    nc = bacc.Bacc("TRN2", target_bir_lowering=False, debug=False)

    o16 = nc.dram_tensor("o16", [NT, D], BF16, kind="ExternalInput")
    blk_a = nc.dram_tensor("blk_a", [R, D], F32, kind="ExternalInput")
    blk_b = nc.dram_tensor("blk_b", [R, D], F32, kind="ExternalInput")
    a16 = nc.dram_tensor("a16", [R, D], BF16, kind="ExternalInput")
    partial = nc.dram_tensor("partial", [1, MT], F32, kind="ExternalOutput")

    with TileContext(nc) as tc:
        with (
            tc.tile_pool(name="const", bufs=1) as constp,
            tc.tile_pool(name="btp", bufs=1) as btp,
            tc.tile_pool(name="smallp", bufs=1) as smallp,
            tc.tile_pool(name="scrp", bufs=2) as scrp,
        ):
            ones = constp.tile([P, 1], F32)
            nc.vector.memset(ones, 1.0)

            # warm up the ACT exp table while the transposes run
            warm = constp.tile([P, 1], F32)
            nc.scalar.activation(warm, ones, mybir.ActivationFunctionType.Exp)

            # out.T in SBUF via DMA-xbar transposes, alternating between the
            # two HWDGE rings (SP / ACT) so the critical first ones overlap
            BT = [btp.tile([P, NT], BF16, name=f"bt{k}") for k in range(KCH)]
            AT = [btp.tile([P, R], BF16, name=f"at{k}") for k in range(KCH)]
            rings = [nc.sync, nc.scalar]
            for k in range(KCH):
                rings[k % 2].dma_start_transpose(
                    AT[k], a16.ap()[:, k * P : (k + 1) * P]
                )
            for g in range(NG):
                for k in range(KCH):
                    rings[k % 2].dma_start_transpose(
                        BT[k][:, g * GRP : (g + 1) * GRP],
                        o16.ap()[g * GRP : (g + 1) * GRP, k * P : (k + 1) * P],
                    )

            # own rows, f32 natural (diag/pos precision)
            blkA = smallp.tile([P, MT, D], F32)
            nc.sync.dma_start(blkA, blk_a.ap().rearrange("(t p) d -> p t d", p=P))
            blkB = smallp.tile([P, MT, D], F32)
            nc.sync.dma_start(blkB, blk_b.ap().rearrange("(t p) d -> p t d", p=P))

            # ssq = sum(a*a), poss = 2*sum(a*b)  (both on DVE; ACT is the
            # bottleneck engine so keep it clear of prologue work)
            ssq = smallp.tile([P, MT], F32)
            poss = smallp.tile([P, MT], F32)
            for t in range(MT):
                sq_scr = scrp.tile([P, D], F32, tag="sq_scr")
                nc.vector.scalar_tensor_tensor(
                    sq_scr, blkA[:, t], 1.0, blkA[:, t],
                    mybir.AluOpType.mult, mybir.AluOpType.mult,
                    accum_out=ssq[:, t : t + 1],
                )
                st_scr = scrp.tile([P, D], F32, tag="st_scr")
                nc.vector.scalar_tensor_tensor(
                    st_scr, blkA[:, t], 2.0, blkB[:, t],
                    mybir.AluOpType.mult, mybir.AluOpType.mult,
                    accum_out=poss[:, t : t + 1],
                )

            # diag = exp(2*ssq) early, while the exp table is resident and
            # ACT is not yet saturated
            diag = smallp.tile([P, MT], F32)
            nc.scalar.activation(
                diag, ssq, mybir.ActivationFunctionType.Exp, scale=2.0
            )

            rowsum = smallp.tile([P, MT * NG], F32)
            nc.vector.memset(rowsum, 0.0)

            # main loop: bf16 matmuls into f32 psum, exp row-sums on ACT
            with tc.tile_pool(name="mps", bufs=2, space="PSUM") as mps:
                for g in range(NG):
                    for m in range(MT):
                        pt = mps.tile([P, GRP], F32, tag="pmm", name=f"pt_{g}_{m}")
                        for k in range(KCH):
                            for j in range(JPG):
                                n0 = (g * JPG + j) * NBLK
                                nc.tensor.matmul(
                                    pt[:, j * NBLK : (j + 1) * NBLK],
                                    AT[k][:, m * P : (m + 1) * P],
                                    BT[k][:, n0 : n0 + NBLK],
                                    start=(k == 0),
                                    stop=(k == KCH - 1),
                                )
                        ex_scr = scrp.tile([P, GRP], F32, tag="ex_scr")
                        nc.scalar.activation(
                            ex_scr, pt, mybir.ActivationFunctionType.Exp,
                            scale=2.0,
                            accum_out=rowsum[:, m * NG + g : m * NG + g + 1],
                        )

            # finalize loss partials
            rs = smallp.tile([P, MT], F32)
            rs3 = rowsum.rearrange("p (m g) -> p m g", g=NG)
            nc.vector.tensor_reduce(
                rs, rs3, mybir.AxisListType.X, mybir.AluOpType.add
            )
            rsd = smallp.tile([P, MT], F32)
            nc.vector.tensor_tensor(rsd, rs, diag, mybir.AluOpType.subtract)
            lg = smallp.tile([P, MT], F32)
            nc.scalar.activation(lg, rsd, mybir.ActivationFunctionType.Ln)
            lossT = smallp.tile([P, MT], F32)
            nc.vector.tensor_tensor(lossT, lg, poss, mybir.AluOpType.subtract)

            with tc.tile_pool(name="fps", bufs=1, space="PSUM") as fps:
                fp = fps.tile([1, MT], F32)
                nc.tensor.matmul(fp, ones, lossT, start=True, stop=True)
                outsb = smallp.tile([1, MT], F32)
                nc.vector.tensor_copy(outsb, fp)
                nc.sync.dma_start(partial.ap(), outsb)

    nc.compile()
    return nc


def _get_nc():
    if "nc" not in _CACHE:
        _CACHE["nc"] = _build()
    return _CACHE["nc"]


def _make_in_maps(o1, o2):
    o16 = np.ascontiguousarray(
        np.concatenate([o1, o2], axis=0).astype(ml_dtypes.bfloat16)
    )
    in_maps = []
    for c in range(NCORES):
        if c < NCORES // 2:
            a = o1[c * R : (c + 1) * R]
            b = o2[c * R : (c + 1) * R]
        else:
            cc = c - NCORES // 2
            a = o2[cc * R : (cc + 1) * R]
            b = o1[cc * R : (cc + 1) * R]
        a = np.ascontiguousarray(a)
        in_maps.append(
            {
                "o16": o16,
                "blk_a": a,
                "blk_b": np.ascontiguousarray(b),
                "a16": np.ascontiguousarray(a.astype(ml_dtypes.bfloat16)),
            }
        )
    return in_maps


def kernel(out_1, out_2, batch_size, **kwargs):
    o1 = np.ascontiguousarray(np.asarray(out_1, dtype=np.float32))
    o2 = np.ascontiguousarray(np.asarray(out_2, dtype=np.float32))
    assert o1.shape == (B, D) and o2.shape == (B, D)
    assert int(batch_size) == B

    nc = _get_nc()
    in_maps = _make_in_maps(o1, o2)
    res = run_bass_kernel_spmd(nc, in_maps, list(range(NCORES)))
    total = np.float64(0.0)
    for c in range(NCORES):
        total += np.float64(res.results[c]["partial"].astype(np.float64).sum())
    return np.float32(total / NT)


# revision 10
# speedup vs baseline: 1.0345x; 1.0345x over previous
"""NT-Xent (SimCLR) contrastive loss kernel for Trainium2, 8 NeuronCores.

Reference computation (B=4096, D=256, T=0.5):
    out  = concat(out_1, out_2)              # [8192, 256]
    sim  = exp(out @ out.T / T)              # [8192, 8192]
    diag = exp(sum(out*out, -1) / T)
    row_sum = sim.sum(-1) - diag
    pos  = exp(sum(out_1*out_2, -1) / T), duplicated
    loss = mean(-log(pos / row_sum)) = mean(log(row_sum) - 2*sum(out_1*out_2, -1))

Sharding: data-parallel over the 8192 rows of sim; core c owns rows
[c*1024, (c+1)*1024). Host-side prep (part of the sharding/layout
strategy): concatenate out -> bf16 copy `o16` plus each core's own f32 row
block and its positive-pair block. Each core then:
  1. builds the full out.T [256, 8192] bf16 in SBUF with DMA-xbar
     transposes (no tensor-engine or gpsimd time),
  2. computes its row-block scores with bf16 matmuls into f32 PSUM,
  3. applies exp(2x) on the scalar engine with fused row-sum accumulation,
  4. computes diag/pos in f32 from its natural-layout blocks and reduces
     its local loss partial with a ones-matmul.
The host sums the 8 partial outputs.

Numerics: row norms ||out_i||^2 ~ 256, so diag = exp(~512) = inf in f32 and
row_sum = inf - inf = nan -> loss = nan, exactly as the reference produces
(HW-verified: ACT exp -> inf, DVE inf-inf -> nan, ACT ln(nan) -> nan). The
bf16 score matmul cannot disturb the nan path; diag/pos terms are computed
in f32.
"""

import os
import sys

for _p in ("/opt/trn_rl_repo", "/root/.axon_site/_ro/trn_rl_repo"):
    if os.path.isdir(_p) and _p not in sys.path:
        sys.path.insert(0, _p)

import ml_dtypes
import numpy as np

import concourse.bass as bass
import concourse.mybir as mybir
from concourse import bacc
from concourse.bass_utils import run_bass_kernel_spmd
from concourse.tile import TileContext

P = 128
D = 256
B = 4096
NT = 2 * B  # 8192 total rows
NCORES = 8
R = NT // NCORES  # 1024 rows per core
MT = R // P  # 8 m-tiles per core
KCH = D // P  # 2 contraction chunks
GRP = 2048  # psum group width (4 banks f32)
NG = NT // GRP  # 4 groups
NBLK = 512  # matmul free dim
JPG = GRP // NBLK  # 4 matmul blocks per group
F32 = mybir.dt.float32
BF16 = mybir.dt.bfloat16

_CACHE: dict = {}


def _build():
    nc = bacc.Bacc("TRN2", target_bir_lowering=False, debug=False)

    o16 = nc.dram_tensor("o16", [NT, D], BF16, kind="ExternalInput")
    blk_a = nc.dram_tensor("blk_a", [R, D], F32, kind="ExternalInput")
    blk_b = nc.dram_tensor("blk_b", [R, D], F32, kind="ExternalInput")
    a16 = nc.dram_tensor("a16", [R, D], BF16, kind="ExternalInput")
    partial = nc.dram_tensor("partial", [1, MT], F32, kind="ExternalOutput")

    with TileContext(nc) as tc:
        with (
            tc.tile_pool(name="const", bufs=1) as constp,
            tc.tile_pool(name="btp", bufs=1) as btp,
            tc.tile_pool(name="smallp", bufs=1) as smallp,
            tc.tile_pool(name="scrp", bufs=2) as scrp,
        ):
            ones = constp.tile([P, 1], F32)
            nc.vector.memset(ones, 1.0)

            # warm up the ACT exp table while the transposes run
            warm = constp.tile([P, 1], F32)
            nc.scalar.activation(warm, ones, mybir.ActivationFunctionType.Exp)

            # out.T in SBUF via DMA-xbar transposes, alternating between the
            # two HWDGE rings (SP / ACT) so the critical first ones overlap
            BT = [btp.tile([P, NT], BF16, name=f"bt{k}") for k in range(KCH)]
            AT = [btp.tile([P, R], BF16, name=f"at{k}") for k in range(KCH)]
            for k in range(KCH):
                nc.sync.dma_start_transpose(AT[k], a16.ap()[:, k * P : (k + 1) * P])
            for g in range(NG):
                for k in range(KCH):
                    nc.sync.dma_start_transpose(
                        BT[k][:, g * GRP : (g + 1) * GRP],
                        o16.ap()[g * GRP : (g + 1) * GRP, k * P : (k + 1) * P],
                    )

            # own rows, f32 natural (diag/pos precision)
            blkA = smallp.tile([P, MT, D], F32)
            nc.sync.dma_start(blkA, blk_a.ap().rearrange("(t p) d -> p t d", p=P))
            blkB = smallp.tile([P, MT, D], F32)
            nc.sync.dma_start(blkB, blk_b.ap().rearrange("(t p) d -> p t d", p=P))

            # ssq = sum(a*a), poss = 2*sum(a*b)  (both on DVE; ACT is the
            # bottleneck engine so keep it clear of prologue work)
            ssq = smallp.tile([P, MT], F32)
            poss = smallp.tile([P, MT], F32)
            for t in range(MT):
                sq_scr = scrp.tile([P, D], F32, tag="sq_scr")
                nc.vector.scalar_tensor_tensor(
                    sq_scr, blkA[:, t], 1.0, blkA[:, t],
                    mybir.AluOpType.mult, mybir.AluOpType.mult,
                    accum_out=ssq[:, t : t + 1],
                )
                st_scr = scrp.tile([P, D], F32, tag="st_scr")
                nc.vector.scalar_tensor_tensor(
                    st_scr, blkA[:, t], 2.0, blkB[:, t],
                    mybir.AluOpType.mult, mybir.AluOpType.mult,
                    accum_out=poss[:, t : t + 1],
                )

            # diag = exp(2*ssq) early, while the exp table is resident and
            # ACT is not yet saturated
            diag = smallp.tile([P, MT], F32)
            nc.scalar.activation(
                diag, ssq, mybir.ActivationFunctionType.Exp, scale=2.0
            )

            rowsum = smallp.tile([P, MT * NG], F32)
            nc.vector.memset(rowsum, 0.0)

            # main loop: bf16 matmuls into f32 psum, exp row-sums on ACT
            with tc.tile_pool(name="mps", bufs=2, space="PSUM") as mps:
                for g in range(NG):
                    for m in range(MT):
                        pt = mps.tile([P, GRP], F32, tag="pmm", name=f"pt_{g}_{m}")
                        for k in range(KCH):
                            for j in range(JPG):
                                n0 = (g * JPG + j) * NBLK
                                nc.tensor.matmul(
                                    pt[:, j * NBLK : (j + 1) * NBLK],
                                    AT[k][:, m * P : (m + 1) * P],
                                    BT[k][:, n0 : n0 + NBLK],
                                    start=(k == 0),
                                    stop=(k == KCH - 1),
                                )
                        ex_scr = scrp.tile([P, GRP], F32, tag="ex_scr")
                        nc.scalar.activation(
                            ex_scr, pt, mybir.ActivationFunctionType.Exp,
                            scale=2.0,
                            accum_out=rowsum[:, m * NG + g : m * NG + g + 1],
                        )

            # finalize loss partials
            rs = smallp.tile([P, MT], F32)
            rs3 = rowsum.rearrange("p (m g) -> p m g", g=NG)
            nc.vector.tensor_reduce(
                rs, rs3, mybir.AxisListType.X, mybir.AluOpType.add
            )
            rsd = smallp.tile([P, MT], F32)
            nc.vector.tensor_tensor(rsd, rs, diag, mybir.AluOpType.subtract)
            lg = smallp.tile([P, MT], F32)
            nc.scalar.activation(lg, rsd, mybir.ActivationFunctionType.Ln)
            lossT = smallp.tile([P, MT], F32)
            nc.vector.tensor_tensor(lossT, lg, poss, mybir.AluOpType.subtract)

            with tc.tile_pool(name="fps", bufs=1, space="PSUM") as fps:
                fp = fps.tile([1, MT], F32)
                nc.tensor.matmul(fp, ones, lossT, start=True, stop=True)
                outsb = smallp.tile([1, MT], F32)
                nc.vector.tensor_copy(outsb, fp)
                nc.sync.dma_start(partial.ap(), outsb)

    nc.compile()
    return nc


def _get_nc():
    if "nc" not in _CACHE:
        _CACHE["nc"] = _build()
    return _CACHE["nc"]


def _make_in_maps(o1, o2):
    o16 = np.ascontiguousarray(
        np.concatenate([o1, o2], axis=0).astype(ml_dtypes.bfloat16)
    )
    in_maps = []
    for c in range(NCORES):
        if c < NCORES // 2:
            a = o1[c * R : (c + 1) * R]
            b = o2[c * R : (c + 1) * R]
        else:
            cc = c - NCORES // 2
            a = o2[cc * R : (cc + 1) * R]
            b = o1[cc * R : (cc + 1) * R]
        a = np.ascontiguousarray(a)
        in_maps.append(
            {
                "o16": o16,
                "blk_a": a,
                "blk_b": np.ascontiguousarray(b),
                "a16": np.ascontiguousarray(a.astype(ml_dtypes.bfloat16)),
            }
        )
    return in_maps


def kernel(out_1, out_2, batch_size, **kwargs):
    o1 = np.ascontiguousarray(np.asarray(out_1, dtype=np.float32))
    o2 = np.ascontiguousarray(np.asarray(out_2, dtype=np.float32))
    assert o1.shape == (B, D) and o2.shape == (B, D)
    assert int(batch_size) == B

    nc = _get_nc()
    in_maps = _make_in_maps(o1, o2)
    res = run_bass_kernel_spmd(nc, in_maps, list(range(NCORES)))
    total = np.float64(0.0)
    for c in range(NCORES):
        total += np.float64(res.results[c]["partial"].astype(np.float64).sum())
    return np.float32(total / NT)


# revision 17
# speedup vs baseline: 1.1318x; 1.0941x over previous
"""NT-Xent (SimCLR) contrastive loss kernel for Trainium2, 8 NeuronCores.

Reference computation (B=4096, D=256, T=0.5):
    out  = concat(out_1, out_2)              # [8192, 256]
    sim  = exp(out @ out.T / T)              # [8192, 8192]
    diag = exp(sum(out*out, -1) / T)
    row_sum = sim.sum(-1) - diag
    pos  = exp(sum(out_1*out_2, -1) / T), duplicated
    loss = mean(-log(pos / row_sum)) = mean(log(row_sum) - 2*sum(out_1*out_2, -1))

Sharding: data-parallel over the 8192 rows of sim; core c owns rows
[c*1024, (c+1)*1024). Host-side prep (part of the sharding/layout
strategy): concatenate out -> bf16 copy `o16` plus each core's own f32 row
block and its positive-pair block. Each core then:
  1. builds the full out.T [256, 8192] bf16 in SBUF with DMA-xbar
     transposes (no tensor-engine or gpsimd time),
  2. computes its row-block scores with bf16 matmuls into f32 PSUM,
  3. applies exp(2x) on the scalar engine with fused row-sum accumulation,
  4. computes diag/pos in f32 from its natural-layout blocks and reduces
     its local loss partial with a ones-matmul.
The host sums the 8 partial outputs.

Numerics: row norms ||out_i||^2 ~ 256, so diag = exp(~512) = inf in f32 and
row_sum = inf - inf = nan -> loss = nan, exactly as the reference produces
(HW-verified: ACT exp -> inf, DVE inf-inf -> nan, ACT ln(nan) -> nan). The
bf16 score matmul cannot disturb the nan path; diag/pos terms are computed
in f32.
"""

import os
import sys

for _p in ("/opt/trn_rl_repo", "/root/.axon_site/_ro/trn_rl_repo"):
    if os.path.isdir(_p) and _p not in sys.path:
        sys.path.insert(0, _p)

import ml_dtypes
import numpy as np

import concourse.bass as bass
import concourse.mybir as mybir
from concourse import bacc
from concourse.bass_utils import run_bass_kernel_spmd
from concourse.tile import TileContext

P = 128
D = 256
B = 4096
NT = 2 * B  # 8192 total rows
NCORES = 8
R = NT // NCORES  # 1024 rows per core
MT = R // P  # 8 m-tiles per core
KCH = D // P  # 2 contraction chunks
GRP = 2048  # psum group width (4 banks f32)
NG = NT // GRP  # 4 groups
NBLK = 512  # matmul free dim
JPG = GRP // NBLK  # 4 matmul blocks per group
F32 = mybir.dt.float32
BF16 = mybir.dt.bfloat16

_CACHE: dict = {}


def _build():
    nc = bacc.Bacc("TRN2", target_bir_lowering=False, debug=False)

    o16 = nc.dram_tensor("o16", [NT, D], BF16, kind="ExternalInput")
    blk_a = nc.dram_tensor("blk_a", [R, D], F32, kind="ExternalInput")
    blk_b = nc.dram_tensor("blk_b", [R, D], F32, kind="ExternalInput")
    a16 = nc.dram_tensor("a16", [R, D], BF16, kind="ExternalInput")
    partial = nc.dram_tensor("partial", [1, MT], F32, kind="ExternalOutput")

    with TileContext(nc) as tc:
        with (
            tc.tile_pool(name="const", bufs=1) as constp,
            tc.tile_pool(name="btp", bufs=1) as btp,
            tc.tile_pool(name="smallp", bufs=1) as smallp,
            tc.tile_pool(name="scrp", bufs=2) as scrp,
        ):
            ones = constp.tile([P, 1], F32)
            nc.vector.memset(ones, 1.0)

            # warm up the ACT exp table while the transposes run
            warm = constp.tile([P, 1], F32)
            nc.scalar.activation(warm, ones, mybir.ActivationFunctionType.Exp)

            # out.T in SBUF via DMA-xbar transposes
            BT = [btp.tile([P, NT], BF16, name=f"bt{k}") for k in range(KCH)]
            AT = [btp.tile([P, R], BF16, name=f"at{k}") for k in range(KCH)]
            # order: everything the first matmuls need (k0) before k1
            nc.sync.dma_start_transpose(AT[0], a16.ap()[:, 0:P])
            nc.sync.dma_start_transpose(
                BT[0][:, 0:GRP], o16.ap()[0:GRP, 0:P]
            )
            nc.sync.dma_start_transpose(AT[1], a16.ap()[:, P : 2 * P])
            nc.sync.dma_start_transpose(
                BT[1][:, 0:GRP], o16.ap()[0:GRP, P : 2 * P]
            )
            for g in range(1, NG):
                for k in range(KCH):
                    nc.sync.dma_start_transpose(
                        BT[k][:, g * GRP : (g + 1) * GRP],
                        o16.ap()[g * GRP : (g + 1) * GRP, k * P : (k + 1) * P],
                    )

            # own rows, f32 natural (diag/pos precision)
            blkA = smallp.tile([P, MT, D], F32)
            nc.sync.dma_start(blkA, blk_a.ap().rearrange("(t p) d -> p t d", p=P))
            blkB = smallp.tile([P, MT, D], F32)
            nc.sync.dma_start(blkB, blk_b.ap().rearrange("(t p) d -> p t d", p=P))

            # ssq = sum(a*a), poss = 2*sum(a*b)  (both on DVE; ACT is the
            # bottleneck engine so keep it clear of prologue work)
            ssq = smallp.tile([P, MT], F32)
            poss = smallp.tile([P, MT], F32)
            for t in range(MT):
                sq_scr = scrp.tile([P, D], F32, tag="sq_scr")
                nc.vector.scalar_tensor_tensor(
                    sq_scr, blkA[:, t], 1.0, blkA[:, t],
                    mybir.AluOpType.mult, mybir.AluOpType.mult,
                    accum_out=ssq[:, t : t + 1],
                )
                st_scr = scrp.tile([P, D], F32, tag="st_scr")
                nc.vector.scalar_tensor_tensor(
                    st_scr, blkA[:, t], 2.0, blkB[:, t],
                    mybir.AluOpType.mult, mybir.AluOpType.mult,
                    accum_out=poss[:, t : t + 1],
                )

            rowsum = smallp.tile([P, MT * NG], F32)
            nc.vector.memset(rowsum, 0.0)

            # main loop: bf16 matmuls into f32 psum, exp row-sums on ACT
            with tc.tile_pool(name="mps", bufs=2, space="PSUM") as mps:
                for g in range(NG):
                    for m in range(MT):
                        pt = mps.tile([P, GRP], F32, tag="pmm", name=f"pt_{g}_{m}")
                        for k in range(KCH):
                            for j in range(JPG):
                                n0 = (g * JPG + j) * NBLK
                                nc.tensor.matmul(
                                    pt[:, j * NBLK : (j + 1) * NBLK],
                                    AT[k][:, m * P : (m + 1) * P],
                                    BT[k][:, n0 : n0 + NBLK],
                                    start=(k == 0),
                                    stop=(k == KCH - 1),
                                )
                        ex_scr = scrp.tile([P, GRP], F32, tag="ex_scr")
                        nc.scalar.activation(
                            ex_scr, pt, mybir.ActivationFunctionType.Exp,
                            scale=2.0,
                            accum_out=rowsum[:, m * NG + g : m * NG + g + 1],
                        )

            # finalize loss partials
            rs = smallp.tile([P, MT], F32)
            rs3 = rowsum.rearrange("p (m g) -> p m g", g=NG)
            nc.vector.tensor_reduce(
                rs, rs3, mybir.AxisListType.X, mybir.AluOpType.add
            )
            diag = smallp.tile([P, MT], F32)
            nc.scalar.activation(
                diag, ssq, mybir.ActivationFunctionType.Exp, scale=2.0
            )
            rsd = smallp.tile([P, MT], F32)
            nc.vector.tensor_tensor(rsd, rs, diag, mybir.AluOpType.subtract)
            lg = smallp.tile([P, MT], F32)
            nc.scalar.activation(lg, rsd, mybir.ActivationFunctionType.Ln)
            lossT = smallp.tile([P, MT], F32)
            nc.vector.tensor_tensor(lossT, lg, poss, mybir.AluOpType.subtract)

            with tc.tile_pool(name="fps", bufs=1, space="PSUM") as fps:
                fp = fps.tile([1, MT], F32)
                nc.tensor.matmul(fp, ones, lossT, start=True, stop=True)
                outsb = smallp.tile([1, MT], F32)
                nc.vector.tensor_copy(outsb, fp)
                nc.sync.dma_start(partial.ap(), outsb)

    nc.compile()
    return nc


def _get_nc():
    if "nc" not in _CACHE:
        _CACHE["nc"] = _build()
    return _CACHE["nc"]


def _make_in_maps(o1, o2):
    o16 = np.ascontiguousarray(
        np.concatenate([o1, o2], axis=0).astype(ml_dtypes.bfloat16)
    )
    in_maps = []
    for c in range(NCORES):
        if c < NCORES // 2:
            a = o1[c * R : (c + 1) * R]
            b = o2[c * R : (c + 1) * R]
        else:
            cc = c - NCORES // 2
            a = o2[cc * R : (cc + 1) * R]
            b = o1[cc * R : (cc + 1) * R]
        a = np.ascontiguousarray(a)
        in_maps.append(
            {
                "o16": o16,
                "blk_a": a,
                "blk_b": np.ascontiguousarray(b),
                "a16": np.ascontiguousarray(a.astype(ml_dtypes.bfloat16)),
            }
        )
    return in_maps


def kernel(out_1, out_2, batch_size, **kwargs):
    o1 = np.ascontiguousarray(np.asarray(out_1, dtype=np.float32))
    o2 = np.ascontiguousarray(np.asarray(out_2, dtype=np.float32))
    assert o1.shape == (B, D) and o2.shape == (B, D)
    assert int(batch_size) == B

    nc = _get_nc()
    in_maps = _make_in_maps(o1, o2)
    res = run_bass_kernel_spmd(nc, in_maps, list(range(NCORES)))
    total = np.float64(0.0)
    for c in range(NCORES):
        total += np.float64(res.results[c]["partial"].astype(np.float64).sum())
    return np.float32(total / NT)


# revision 18
# speedup vs baseline: 1.1344x; 1.0023x over previous
"""NT-Xent (SimCLR) contrastive loss kernel for Trainium2, 8 NeuronCores.

Reference computation (B=4096, D=256, T=0.5):
    out  = concat(out_1, out_2)              # [8192, 256]
    sim  = exp(out @ out.T / T)              # [8192, 8192]
    diag = exp(sum(out*out, -1) / T)
    row_sum = sim.sum(-1) - diag
    pos  = exp(sum(out_1*out_2, -1) / T), duplicated
    loss = mean(-log(pos / row_sum)) = mean(log(row_sum) - 2*sum(out_1*out_2, -1))

Sharding: data-parallel over the 8192 rows of sim; core c owns rows
[c*1024, (c+1)*1024). Host-side prep (part of the sharding/layout
strategy): concatenate out -> bf16 copy `o16` plus each core's own f32 row
block and its positive-pair block. Each core then:
  1. builds the full out.T [256, 8192] bf16 in SBUF with DMA-xbar
     transposes (no tensor-engine or gpsimd time),
  2. computes its row-block scores with bf16 matmuls into f32 PSUM,
  3. applies exp(2x) on the scalar engine with fused row-sum accumulation,
  4. computes diag/pos in f32 from its natural-layout blocks and reduces
     its local loss partial with a ones-matmul.
The host sums the 8 partial outputs.

Numerics: row norms ||out_i||^2 ~ 256, so diag = exp(~512) = inf in f32 and
row_sum = inf - inf = nan -> loss = nan, exactly as the reference produces
(HW-verified: ACT exp -> inf, DVE inf-inf -> nan, ACT ln(nan) -> nan). The
bf16 score matmul cannot disturb the nan path; diag/pos terms are computed
in f32.
"""

import os
import sys

for _p in ("/opt/trn_rl_repo", "/root/.axon_site/_ro/trn_rl_repo"):
    if os.path.isdir(_p) and _p not in sys.path:
        sys.path.insert(0, _p)

import ml_dtypes
import numpy as np

import concourse.mybir as mybir
from concourse import bacc
from concourse.bass_utils import run_bass_kernel_spmd
from concourse.tile import TileContext

P = 128
D = 256
B = 4096
NT = 2 * B  # 8192 total rows
NCORES = 8
R = NT // NCORES  # 1024 rows per core
MT = R // P  # 8 m-tiles per core
KCH = D // P  # 2 contraction chunks
GRP = 2048  # psum group width (4 banks f32)
NG = NT // GRP  # 4 groups
NBLK = 512  # matmul free dim
JPG = GRP // NBLK  # 4 matmul blocks per group
F32 = mybir.dt.float32
BF16 = mybir.dt.bfloat16

_CACHE: dict = {}


def _build():
    nc = bacc.Bacc("TRN2", target_bir_lowering=False, debug=False)

    o16 = nc.dram_tensor("o16", [NT, D], BF16, kind="ExternalInput")
    blk_a = nc.dram_tensor("blk_a", [R, D], F32, kind="ExternalInput")
    blk_b = nc.dram_tensor("blk_b", [R, D], F32, kind="ExternalInput")
    a16 = nc.dram_tensor("a16", [R, D], BF16, kind="ExternalInput")
    partial = nc.dram_tensor("partial", [1, MT], F32, kind="ExternalOutput")

    with TileContext(nc) as tc:
        with (
            tc.tile_pool(name="const", bufs=1) as constp,
            tc.tile_pool(name="btp", bufs=1) as btp,
            tc.tile_pool(name="smallp", bufs=1) as smallp,
            tc.tile_pool(name="scrp", bufs=2) as scrp,
        ):
            ones = constp.tile([P, 1], F32)
            nc.vector.memset(ones, 1.0)

            # warm up the ACT exp table while the transposes run
            warm = constp.tile([P, 1], F32)
            nc.scalar.activation(warm, ones, mybir.ActivationFunctionType.Exp)

            # out.T in SBUF via DMA-xbar transposes
            BT = [btp.tile([P, NT], BF16, name=f"bt{k}") for k in range(KCH)]
            AT = [btp.tile([P, R], BF16, name=f"at{k}") for k in range(KCH)]
            # order: everything the first matmuls need (k0) before k1
            nc.sync.dma_start_transpose(AT[0], a16.ap()[:, 0:P])
            nc.sync.dma_start_transpose(
                BT[0][:, 0:GRP], o16.ap()[0:GRP, 0:P]
            )
            nc.sync.dma_start_transpose(AT[1], a16.ap()[:, P : 2 * P])
            nc.sync.dma_start_transpose(
                BT[1][:, 0:GRP], o16.ap()[0:GRP, P : 2 * P]
            )
            for g in range(1, NG):
                for k in range(KCH):
                    nc.sync.dma_start_transpose(
                        BT[k][:, g * GRP : (g + 1) * GRP],
                        o16.ap()[g * GRP : (g + 1) * GRP, k * P : (k + 1) * P],
                    )

            # own rows, f32 natural (diag/pos precision)
            blkA = smallp.tile([P, MT, D], F32)
            nc.sync.dma_start(blkA, blk_a.ap().rearrange("(t p) d -> p t d", p=P))
            blkB = smallp.tile([P, MT, D], F32)
            nc.sync.dma_start(blkB, blk_b.ap().rearrange("(t p) d -> p t d", p=P))

            # ssq = sum(a*a), poss = 2*sum(a*b)  (both on DVE; ACT is the
            # bottleneck engine so keep it clear of prologue work)
            ssq = smallp.tile([P, MT], F32)
            poss = smallp.tile([P, MT], F32)
            for t in range(MT):
                sq_scr = scrp.tile([P, D], F32, tag="sq_scr")
                nc.vector.scalar_tensor_tensor(
                    sq_scr, blkA[:, t], 1.0, blkA[:, t],
                    mybir.AluOpType.mult, mybir.AluOpType.mult,
                    accum_out=ssq[:, t : t + 1],
                )
                st_scr = scrp.tile([P, D], F32, tag="st_scr")
                nc.vector.scalar_tensor_tensor(
                    st_scr, blkA[:, t], 2.0, blkB[:, t],
                    mybir.AluOpType.mult, mybir.AluOpType.mult,
                    accum_out=poss[:, t : t + 1],
                )

            rowsum = smallp.tile([P, MT * NG], F32)
            nc.vector.memset(rowsum, 0.0)

            # main loop: bf16 matmuls into f32 psum, exp row-sums on ACT
            with tc.tile_pool(name="mps", bufs=2, space="PSUM") as mps:
                for g in range(NG):
                    for m in range(MT):
                        pt = mps.tile([P, GRP], F32, tag="pmm", name=f"pt_{g}_{m}")
                        for k in range(KCH):
                            for j in range(JPG):
                                n0 = (g * JPG + j) * NBLK
                                nc.tensor.matmul(
                                    pt[:, j * NBLK : (j + 1) * NBLK],
                                    AT[k][:, m * P : (m + 1) * P],
                                    BT[k][:, n0 : n0 + NBLK],
                                    start=(k == 0),
                                    stop=(k == KCH - 1),
                                )
                        ex_scr = scrp.tile([P, GRP], F32, tag="ex_scr")
                        nc.scalar.activation(
                            ex_scr, pt, mybir.ActivationFunctionType.Exp,
                            scale=2.0,
                            accum_out=rowsum[:, m * NG + g : m * NG + g + 1],
                        )

            # finalize loss partials
            rs = smallp.tile([P, MT], F32)
            rs3 = rowsum.rearrange("p (m g) -> p m g", g=NG)
            nc.vector.tensor_reduce(
                rs, rs3, mybir.AxisListType.X, mybir.AluOpType.add
            )
            diag = smallp.tile([P, MT], F32)
            nc.scalar.activation(
                diag, ssq, mybir.ActivationFunctionType.Exp, scale=2.0
            )
            rsd = smallp.tile([P, MT], F32)
            nc.vector.tensor_tensor(rsd, rs, diag, mybir.AluOpType.subtract)
            lg = smallp.tile([P, MT], F32)
            nc.scalar.activation(lg, rsd, mybir.ActivationFunctionType.Ln)
            lossT = smallp.tile([P, MT], F32)
            nc.vector.tensor_tensor(lossT, lg, poss, mybir.AluOpType.subtract)

            with tc.tile_pool(name="fps", bufs=1, space="PSUM") as fps:
                fp = fps.tile([1, MT], F32)
                nc.tensor.matmul(fp, ones, lossT, start=True, stop=True)
                outsb = smallp.tile([1, MT], F32)
                nc.vector.tensor_copy(outsb, fp)
                nc.sync.dma_start(partial.ap(), outsb)

    nc.compile()
    return nc


def _get_nc():
    if "nc" not in _CACHE:
        _CACHE["nc"] = _build()
    return _CACHE["nc"]


def _make_in_maps(o1, o2):
    o16 = np.ascontiguousarray(
        np.concatenate([o1, o2], axis=0).astype(ml_dtypes.bfloat16)
    )
    in_maps = []
    for c in range(NCORES):
        if c < NCORES // 2:
            a = o1[c * R : (c + 1) * R]
            b = o2[c * R : (c + 1) * R]
        else:
            cc = c - NCORES // 2
            a = o2[cc * R : (cc + 1) * R]
            b = o1[cc * R : (cc + 1) * R]
        a = np.ascontiguousarray(a)
        in_maps.append(
            {
                "o16": o16,
                "blk_a": a,
                "blk_b": np.ascontiguousarray(b),
                "a16": np.ascontiguousarray(a.astype(ml_dtypes.bfloat16)),
            }
        )
    return in_maps


def kernel(out_1, out_2, batch_size, **kwargs):
    o1 = np.ascontiguousarray(np.asarray(out_1, dtype=np.float32))
    o2 = np.ascontiguousarray(np.asarray(out_2, dtype=np.float32))
    assert o1.shape == (B, D) and o2.shape == (B, D)
    assert int(batch_size) == B

    nc = _get_nc()
    in_maps = _make_in_maps(o1, o2)
    res = run_bass_kernel_spmd(nc, in_maps, list(range(NCORES)))
    total = np.float64(0.0)
    for c in range(NCORES):
        total += np.float64(res.results[c]["partial"].astype(np.float64).sum())
    return np.float32(total / NT)


# revision 23
# speedup vs baseline: 1.1585x; 1.0212x over previous
"""NT-Xent (SimCLR) contrastive loss kernel for Trainium2, 8 NeuronCores.

Reference computation (B=4096, D=256, T=0.5):
    out  = concat(out_1, out_2)              # [8192, 256]
    sim  = exp(out @ out.T / T)              # [8192, 8192]
    diag = exp(sum(out*out, -1) / T)
    row_sum = sim.sum(-1) - diag
    pos  = exp(sum(out_1*out_2, -1) / T), duplicated
    loss = mean(-log(pos / row_sum)) = mean(log(row_sum) - 2*sum(out_1*out_2, -1))

Sharding: data-parallel over the 8192 rows of sim; core c owns rows
[c*1024, (c+1)*1024). Host-side prep (part of the sharding/layout
strategy): concatenate out -> bf16 copy `o16` plus each core's own f32 row
block and its positive-pair block. Each core then:
  1. builds the full out.T [256, 8192] bf16 in SBUF with DMA-xbar
     transposes (no tensor-engine or gpsimd time),
  2. computes its row-block scores with bf16 matmuls into f32 PSUM,
  3. applies exp(2x) on the scalar engine with fused row-sum accumulation,
  4. computes diag/pos in f32 from its natural-layout blocks and reduces
     its local loss partial with a ones-matmul.
The host sums the 8 partial outputs.

Numerics: row norms ||out_i||^2 ~ 256, so diag = exp(~512) = inf in f32 and
row_sum = inf - inf = nan -> loss = nan, exactly as the reference produces
(HW-verified: ACT exp -> inf, DVE inf-inf -> nan, ACT ln(nan) -> nan). The
bf16 score matmul cannot disturb the nan path; diag/pos terms are computed
in f32.
"""

import os
import sys

for _p in ("/opt/trn_rl_repo", "/root/.axon_site/_ro/trn_rl_repo"):
    if os.path.isdir(_p) and _p not in sys.path:
        sys.path.insert(0, _p)

import ml_dtypes
import numpy as np

import concourse.mybir as mybir
from concourse import bacc
from concourse.bass_utils import run_bass_kernel_spmd
from concourse.tile import TileContext

P = 128
D = 256
B = 4096
NT = 2 * B  # 8192 total rows
NCORES = 8
R = NT // NCORES  # 1024 rows per core
MT = R // P  # 8 m-tiles per core
KCH = D // P  # 2 contraction chunks
GRP = 2048  # psum group width (4 banks f32)
NG = NT // GRP  # 4 groups
NBLK = 512  # matmul free dim
JPG = GRP // NBLK  # 4 matmul blocks per group
F32 = mybir.dt.float32
BF16 = mybir.dt.bfloat16

_CACHE: dict = {}


def _patch_act_tables():
    """Force Exp and Ln onto the combined `natural_log_exp_and_others` ACT
    table set so the kernel needs a single ACT_TABLE_LOAD instead of two
    (the second load + pipe drain otherwise sits on the critical tail).
    Indices (act_func_set_id) are preserved; Exp/Ln are just removed from
    the other sets so the selection pass has one choice."""
    if _CACHE.get("tables_patched"):
        return
    import concourse.hw_specs as hw_specs

    orig = hw_specs.get_activation_tables

    def patched(module_arch):
        tabs = {k: set(v) for k, v in orig(module_arch).items()}
        both = {
            mybir.ActivationFunctionType.Exp,
            mybir.ActivationFunctionType.Ln,
        }
        if "natural_log_exp_and_others" in tabs and both <= tabs[
            "natural_log_exp_and_others"
        ]:
            for name, s in tabs.items():
                if name != "natural_log_exp_and_others":
                    s -= both
        return tabs

    bacc.get_activation_tables = patched
    _CACHE["tables_patched"] = True


def _build():
    _patch_act_tables()
    nc = bacc.Bacc("TRN2", target_bir_lowering=False, debug=False)

    o16 = nc.dram_tensor("o16", [NT, D], BF16, kind="ExternalInput")
    blk_a = nc.dram_tensor("blk_a", [R, D], F32, kind="ExternalInput")
    blk_b = nc.dram_tensor("blk_b", [R, D], F32, kind="ExternalInput")
    a16 = nc.dram_tensor("a16", [R, D], BF16, kind="ExternalInput")
    partial = nc.dram_tensor("partial", [1, MT], F32, kind="ExternalOutput")

    with TileContext(nc) as tc:
        with (
            tc.tile_pool(name="const", bufs=1) as constp,
            tc.tile_pool(name="btp", bufs=1) as btp,
            tc.tile_pool(name="smallp", bufs=1) as smallp,
            tc.tile_pool(name="scrp", bufs=2) as scrp,
        ):
            ones = constp.tile([P, 1], F32)
            nc.vector.memset(ones, 1.0)

            # warm up the PE (IRAM fetch + sequencer) with a dummy matmul so
            # the first real matmul isn't stuck behind a ~4us cold ifetch
            ones16 = constp.tile([P, 1], BF16)
            nc.vector.memset(ones16, 1.0)
            with tc.tile_pool(name="wps", bufs=1, space="PSUM") as wps:
                wp = wps.tile([1, 1], F32)
                nc.tensor.matmul(wp, ones16, ones16, start=True, stop=True)

            # out.T in SBUF via DMA-xbar transposes
            BT = [btp.tile([P, NT], BF16, name=f"bt{k}") for k in range(KCH)]
            AT = [btp.tile([P, R], BF16, name=f"at{k}") for k in range(KCH)]
            # order: everything the first matmuls need (k0) before k1. The
            # two small AT transposes ride the scalar HWDGE ring — it's idle
            # this early (ACT only starts exp'ing at ~16us), and that lets
            # the critical BT g0 transposes start immediately on sync.
            nc.scalar.dma_start_transpose(AT[0], a16.ap()[:, 0:P])
            nc.scalar.dma_start_transpose(AT[1], a16.ap()[:, P : 2 * P])
            nc.sync.dma_start_transpose(
                BT[0][:, 0:GRP], o16.ap()[0:GRP, 0:P]
            )
            nc.sync.dma_start_transpose(
                BT[1][:, 0:GRP], o16.ap()[0:GRP, P : 2 * P]
            )
            for g in range(1, NG):
                for k in range(KCH):
                    nc.sync.dma_start_transpose(
                        BT[k][:, g * GRP : (g + 1) * GRP],
                        o16.ap()[g * GRP : (g + 1) * GRP, k * P : (k + 1) * P],
                    )

            # warm up the ACT table (combined exp+ln set) while the
            # transposes run — AFTER the transpose dispatches so the table
            # load doesn't delay the scalar-ring DMA issues
            warm = constp.tile([P, 1], F32)
            nc.scalar.activation(warm, ones, mybir.ActivationFunctionType.Exp)

            # own rows, f32 natural (diag/pos precision)
            blkA = smallp.tile([P, MT, D], F32)
            nc.sync.dma_start(blkA, blk_a.ap().rearrange("(t p) d -> p t d", p=P))
            blkB = smallp.tile([P, MT, D], F32)
            nc.sync.dma_start(blkB, blk_b.ap().rearrange("(t p) d -> p t d", p=P))

            # ssq = sum(a*a), poss = 2*sum(a*b)  (both on DVE; ACT is the
            # bottleneck engine so keep it clear of prologue work)
            ssq = smallp.tile([P, MT], F32)
            poss = smallp.tile([P, MT], F32)
            for t in range(MT):
                sq_scr = scrp.tile([P, D], F32, tag="sq_scr")
                nc.vector.scalar_tensor_tensor(
                    sq_scr, blkA[:, t], 1.0, blkA[:, t],
                    mybir.AluOpType.mult, mybir.AluOpType.mult,
                    accum_out=ssq[:, t : t + 1],
                )
                st_scr = scrp.tile([P, D], F32, tag="st_scr")
                nc.vector.scalar_tensor_tensor(
                    st_scr, blkA[:, t], 2.0, blkB[:, t],
                    mybir.AluOpType.mult, mybir.AluOpType.mult,
                    accum_out=poss[:, t : t + 1],
                )

            rowsum = smallp.tile([P, MT * NG], F32)
            nc.vector.memset(rowsum, 0.0)

            # main loop: bf16 matmuls into f32 psum, exp row-sums on ACT
            with tc.tile_pool(name="mps", bufs=2, space="PSUM") as mps:
                for g in range(NG):
                    for m in range(MT):
                        pt = mps.tile([P, GRP], F32, tag="pmm", name=f"pt_{g}_{m}")
                        for k in range(KCH):
                            for j in range(JPG):
                                n0 = (g * JPG + j) * NBLK
                                nc.tensor.matmul(
                                    pt[:, j * NBLK : (j + 1) * NBLK],
                                    AT[k][:, m * P : (m + 1) * P],
                                    BT[k][:, n0 : n0 + NBLK],
                                    start=(k == 0),
                                    stop=(k == KCH - 1),
                                )
                        ex_scr = scrp.tile([P, GRP], F32, tag="ex_scr")
                        nc.scalar.activation(
                            ex_scr, pt, mybir.ActivationFunctionType.Exp,
                            scale=2.0,
                            accum_out=rowsum[:, m * NG + g : m * NG + g + 1],
                        )

            # finalize loss partials
            rs = smallp.tile([P, MT], F32)
            rs3 = rowsum.rearrange("p (m g) -> p m g", g=NG)
            nc.vector.tensor_reduce(
                rs, rs3, mybir.AxisListType.X, mybir.AluOpType.add
            )
            diag = smallp.tile([P, MT], F32)
            nc.scalar.activation(
                diag, ssq, mybir.ActivationFunctionType.Exp, scale=2.0
            )
            rsd = smallp.tile([P, MT], F32)
            nc.vector.tensor_tensor(rsd, rs, diag, mybir.AluOpType.subtract)
            lg = smallp.tile([P, MT], F32)
            nc.scalar.activation(lg, rsd, mybir.ActivationFunctionType.Ln)
            lossT = smallp.tile([P, MT], F32)
            nc.vector.tensor_tensor(lossT, lg, poss, mybir.AluOpType.subtract)

            with tc.tile_pool(name="fps", bufs=1, space="PSUM") as fps:
                fp = fps.tile([1, MT], F32)
                nc.tensor.matmul(fp, ones, lossT, start=True, stop=True)
                outsb = smallp.tile([1, MT], F32)
                nc.vector.tensor_copy(outsb, fp)
                nc.sync.dma_start(partial.ap(), outsb)

    nc.compile()
    return nc


def _get_nc():
    if "nc" not in _CACHE:
        _CACHE["nc"] = _build()
    return _CACHE["nc"]


def _make_in_maps(o1, o2):
    o16 = np.ascontiguousarray(
        np.concatenate([o1, o2], axis=0).astype(ml_dtypes.bfloat16)
    )
    in_maps = []
    for c in range(NCORES):
        if c < NCORES // 2:
            a = o1[c * R : (c + 1) * R]
            b = o2[c * R : (c + 1) * R]
        else:
            cc = c - NCORES // 2
            a = o2[cc * R : (cc + 1) * R]
            b = o1[cc * R : (cc + 1) * R]
        a = np.ascontiguousarray(a)
        in_maps.append(
            {
                "o16": o16,
                "blk_a": a,
                "blk_b": np.ascontiguousarray(b),
                "a16": np.ascontiguousarray(a.astype(ml_dtypes.bfloat16)),
            }
        )
    return in_maps


def kernel(out_1, out_2, batch_size, **kwargs):
    o1 = np.ascontiguousarray(np.asarray(out_1, dtype=np.float32))
    o2 = np.ascontiguousarray(np.asarray(out_2, dtype=np.float32))
    assert o1.shape == (B, D) and o2.shape == (B, D)
    assert int(batch_size) == B

    nc = _get_nc()
    in_maps = _make_in_maps(o1, o2)
    res = run_bass_kernel_spmd(nc, in_maps, list(range(NCORES)))
    total = np.float64(0.0)
    for c in range(NCORES):
        total += np.float64(res.results[c]["partial"].astype(np.float64).sum())
    return np.float32(total / NT)


# revision 33
# speedup vs baseline: 1.1682x; 1.0084x over previous
"""NT-Xent (SimCLR) contrastive loss kernel for Trainium2, 8 NeuronCores.

Reference computation (B=4096, D=256, T=0.5):
    out  = concat(out_1, out_2)              # [8192, 256]
    sim  = exp(out @ out.T / T)              # [8192, 8192]
    diag = exp(sum(out*out, -1) / T)
    row_sum = sim.sum(-1) - diag
    pos  = exp(sum(out_1*out_2, -1) / T), duplicated
    loss = mean(-log(pos / row_sum)) = mean(log(row_sum) - 2*sum(out_1*out_2, -1))

Sharding: data-parallel over the 8192 rows of sim; core c owns rows
[c*1024, (c+1)*1024). Host-side prep (part of the sharding/layout
strategy): concatenate out -> bf16 copy `o16` plus each core's own f32 row
block and its positive-pair block. Each core then:
  1. builds the full out.T [256, 8192] bf16 in SBUF with DMA-xbar
     transposes (no tensor-engine or gpsimd time),
  2. computes its row-block scores with bf16 matmuls into f32 PSUM,
  3. applies exp(2x) on the scalar engine with fused row-sum accumulation,
  4. computes diag/pos in f32 from its natural-layout blocks and reduces
     its local loss partial with a ones-matmul.
The host sums the 8 partial outputs.

Numerics: row norms ||out_i||^2 ~ 256, so diag = exp(~512) = inf in f32 and
row_sum = inf - inf = nan -> loss = nan, exactly as the reference produces
(HW-verified: ACT exp -> inf, DVE inf-inf -> nan, ACT ln(nan) -> nan). The
bf16 score matmul cannot disturb the nan path; diag/pos terms are computed
in f32.
"""

import os
import sys

for _p in ("/opt/trn_rl_repo", "/root/.axon_site/_ro/trn_rl_repo"):
    if os.path.isdir(_p) and _p not in sys.path:
        sys.path.insert(0, _p)

import ml_dtypes
import numpy as np

import concourse.mybir as mybir
from concourse import bacc
from concourse.bass_utils import run_bass_kernel_spmd
from concourse.tile import TileContext

P = 128
D = 256
B = 4096
NT = 2 * B  # 8192 total rows
NCORES = 8
R = NT // NCORES  # 1024 rows per core
MT = R // P  # 8 m-tiles per core
KCH = D // P  # 2 contraction chunks
GRP = 2048  # psum group width (4 banks f32)
NG = NT // GRP  # 4 groups
NBLK = 512  # matmul free dim
JPG = GRP // NBLK  # 4 matmul blocks per group
F32 = mybir.dt.float32
BF16 = mybir.dt.bfloat16

_CACHE: dict = {}


def _patch_act_tables():
    """Force Exp and Ln onto the combined `natural_log_exp_and_others` ACT
    table set so the kernel needs a single ACT_TABLE_LOAD instead of two
    (the second load + pipe drain otherwise sits on the critical tail).
    Indices (act_func_set_id) are preserved; Exp/Ln are just removed from
    the other sets so the selection pass has one choice."""
    if _CACHE.get("tables_patched"):
        return
    import concourse.hw_specs as hw_specs

    orig = hw_specs.get_activation_tables

    def patched(module_arch):
        tabs = {k: set(v) for k, v in orig(module_arch).items()}
        both = {
            mybir.ActivationFunctionType.Exp,
            mybir.ActivationFunctionType.Ln,
        }
        if "natural_log_exp_and_others" in tabs and both <= tabs[
            "natural_log_exp_and_others"
        ]:
            for name, s in tabs.items():
                if name != "natural_log_exp_and_others":
                    s -= both
        return tabs

    bacc.get_activation_tables = patched
    _CACHE["tables_patched"] = True


def _build():
    _patch_act_tables()
    nc = bacc.Bacc("TRN2", target_bir_lowering=False, debug=False)

    o16 = nc.dram_tensor("o16", [NT, D], BF16, kind="ExternalInput")
    blk_a = nc.dram_tensor("blk_a", [R, D], F32, kind="ExternalInput")
    blk_b = nc.dram_tensor("blk_b", [R, D], F32, kind="ExternalInput")
    a16 = nc.dram_tensor("a16", [R, D], BF16, kind="ExternalInput")
    partial = nc.dram_tensor("partial", [1, MT], F32, kind="ExternalOutput")

    with TileContext(nc) as tc:
        with (
            tc.tile_pool(name="const", bufs=1) as constp,
            tc.tile_pool(name="btp", bufs=1) as btp,
            tc.tile_pool(name="smallp", bufs=1) as smallp,
            tc.tile_pool(name="scrp", bufs=2) as scrp,
        ):
            ones = constp.tile([P, 1], F32)
            nc.vector.memset(ones, 1.0)

            # warm up the PE (IRAM fetch + sequencer) with a dummy matmul so
            # the first real matmul isn't stuck behind a ~4us cold ifetch
            ones16 = constp.tile([P, 1], BF16)
            nc.vector.memset(ones16, 1.0)
            with tc.tile_pool(name="wps", bufs=1, space="PSUM") as wps:
                wp = wps.tile([1, 1], F32)
                nc.tensor.matmul(wp, ones16, ones16, start=True, stop=True)

            # out.T in SBUF via DMA-xbar transposes
            BT = [btp.tile([P, NT], BF16, name=f"bt{k}") for k in range(KCH)]
            AT = [btp.tile([P, R], BF16, name=f"at{k}") for k in range(KCH)]
            # order: everything the first matmuls need (k0) before k1. The
            # two small AT transposes ride the scalar HWDGE ring — it's idle
            # this early (ACT only starts exp'ing at ~16us), and that lets
            # the critical BT g0 transposes start immediately on sync.
            nc.scalar.dma_start_transpose(AT[0], a16.ap()[:, 0:P])
            nc.scalar.dma_start_transpose(AT[1], a16.ap()[:, P : 2 * P])
            nc.sync.dma_start_transpose(
                BT[0][:, 0:GRP], o16.ap()[0:GRP, 0:P]
            )
            nc.sync.dma_start_transpose(
                BT[1][:, 0:GRP], o16.ap()[0:GRP, P : 2 * P]
            )
            for g in range(1, NG):
                for k in range(KCH):
                    nc.sync.dma_start_transpose(
                        BT[k][:, g * GRP : (g + 1) * GRP],
                        o16.ap()[g * GRP : (g + 1) * GRP, k * P : (k + 1) * P],
                    )

            # warm up the ACT table (combined exp+ln set) while the
            # transposes run — AFTER the transpose dispatches so the table
            # load doesn't delay the scalar-ring DMA issues
            warm = constp.tile([P, 1], F32)
            nc.scalar.activation(warm, ones, mybir.ActivationFunctionType.Exp)

            # own rows, f32 natural (diag/pos precision)
            blkA = smallp.tile([P, MT, D], F32)
            nc.sync.dma_start(blkA, blk_a.ap().rearrange("(t p) d -> p t d", p=P))
            blkB = smallp.tile([P, MT, D], F32)
            nc.sync.dma_start(blkB, blk_b.ap().rearrange("(t p) d -> p t d", p=P))

            # ssq = sum(a*a), poss = 2*sum(a*b)  (both on DVE; ACT is the
            # bottleneck engine so keep it clear of prologue work)
            ssq = smallp.tile([P, MT], F32)
            poss = smallp.tile([P, MT], F32)
            for t in range(MT):
                sq_scr = scrp.tile([P, D], F32, tag="sq_scr")
                nc.vector.scalar_tensor_tensor(
                    sq_scr, blkA[:, t], 1.0, blkA[:, t],
                    mybir.AluOpType.mult, mybir.AluOpType.mult,
                    accum_out=ssq[:, t : t + 1],
                )
                st_scr = scrp.tile([P, D], F32, tag="st_scr")
                nc.vector.scalar_tensor_tensor(
                    st_scr, blkA[:, t], 2.0, blkB[:, t],
                    mybir.AluOpType.mult, mybir.AluOpType.mult,
                    accum_out=poss[:, t : t + 1],
                )

            rowsum = smallp.tile([P, MT * NG], F32)
            nc.vector.memset(rowsum, 0.0)

            # main loop: bf16 matmuls into f32 psum, exp row-sums on ACT
            with tc.tile_pool(name="mps", bufs=2, space="PSUM") as mps:
                for g in range(NG):
                    for m in range(MT):
                        pt = mps.tile([P, GRP], F32, tag="pmm", name=f"pt_{g}_{m}")
                        for k in range(KCH):
                            for j in range(JPG):
                                n0 = (g * JPG + j) * NBLK
                                nc.tensor.matmul(
                                    pt[:, j * NBLK : (j + 1) * NBLK],
                                    AT[k][:, m * P : (m + 1) * P],
                                    BT[k][:, n0 : n0 + NBLK],
                                    start=(k == 0),
                                    stop=(k == KCH - 1),
                                )
                        ex_scr = scrp.tile([P, GRP], F32, tag="ex_scr")
                        nc.scalar.activation(
                            ex_scr, pt, mybir.ActivationFunctionType.Exp,
                            scale=2.0,
                            accum_out=rowsum[:, m * NG + g : m * NG + g + 1],
                        )

            # finalize loss partials
            rs = smallp.tile([P, MT], F32)
            rsv = rowsum.rearrange("p (m g) -> p m g", g=NG)
            nc.vector.tensor_reduce(
                rs, rsv, mybir.AxisListType.X, mybir.AluOpType.add
            )
            diag = smallp.tile([P, MT], F32)
            nc.scalar.activation(
                diag, ssq, mybir.ActivationFunctionType.Exp, scale=2.0
            )
            rsd = smallp.tile([P, MT], F32)
            nc.vector.tensor_tensor(rsd, rs, diag, mybir.AluOpType.subtract)
            lg = smallp.tile([P, MT], F32)
            nc.scalar.activation(lg, rsd, mybir.ActivationFunctionType.Ln)
            lossT = smallp.tile([P, MT], F32)
            nc.vector.tensor_tensor(lossT, lg, poss, mybir.AluOpType.subtract)

            with tc.tile_pool(name="fps", bufs=1, space="PSUM") as fps:
                fp = fps.tile([1, MT], F32)
                nc.tensor.matmul(fp, ones, lossT, start=True, stop=True)
                outsb = smallp.tile([1, MT], F32)
                nc.vector.tensor_copy(outsb, fp)
                nc.sync.dma_start(partial.ap(), outsb)

    nc.compile()
    return nc


def _get_nc():
    if "nc" not in _CACHE:
        _CACHE["nc"] = _build()
    return _CACHE["nc"]


def _make_in_maps(o1, o2):
    o16 = np.ascontiguousarray(
        np.concatenate([o1, o2], axis=0).astype(ml_dtypes.bfloat16)
    )
    in_maps = []
    for c in range(NCORES):
        if c < NCORES // 2:
            a = o1[c * R : (c + 1) * R]
            b = o2[c * R : (c + 1) * R]
        else:
            cc = c - NCORES // 2
            a = o2[cc * R : (cc + 1) * R]
            b = o1[cc * R : (cc + 1) * R]
        a = np.ascontiguousarray(a)
        in_maps.append(
            {
                "o16": o16,
                "blk_a": a,
                "blk_b": np.ascontiguousarray(b),
                "a16": np.ascontiguousarray(a.astype(ml_dtypes.bfloat16)),
            }
        )
    return in_maps


def kernel(out_1, out_2, batch_size, **kwargs):
    o1 = np.ascontiguousarray(np.asarray(out_1, dtype=np.float32))
    o2 = np.ascontiguousarray(np.asarray(out_2, dtype=np.float32))
    assert o1.shape == (B, D) and o2.shape == (B, D)
    assert int(batch_size) == B

    nc = _get_nc()
    in_maps = _make_in_maps(o1, o2)
    res = run_bass_kernel_spmd(nc, in_maps, list(range(NCORES)))
    total = np.float64(0.0)
    for c in range(NCORES):
        total += np.float64(res.results[c]["partial"].astype(np.float64).sum())
    return np.float32(total / NT)
